# revision 1
# baseline (speedup 1.0000x reference)
"""Trainium2 Bass kernel for nn_DiffKS (time-varying FIR / InvertLPC forward).

Computes x[b,t] = y[b,t] + sum_k A[b,t,k] * y_padded[b, t+N-1-k]
with B=4, T=64000, N=588, y_padded = concat(flip(zi), y).

Sharding: 8 cores = 4 batches x 2 halves of T (each core: 32000 time steps).
Per-core layout: SBUF partition p owns 250 contiguous time steps
(t = 250*p + I, I in [0,250)).  The shifted-signal operand for output sample
I is a contiguous 588-column slice of a tiny per-partition reversed window
tile v[128, 837] (v[p] = reversed y_padded[250p : 250p+837] of the shard),
so no shift/Hankel materialization is needed on-device.  One fused DVE
tensor_tensor_reduce per output sample does multiply + sum in a single pass,
while A streams from HBM in large contiguous DMAs (memory-bound roofline).
"""

import os
import sys

import numpy as np

for _p in ("/opt/trn_rl_repo", "/opt/pypackages"):
    if _p not in sys.path:
        sys.path.append(_p)

# defensive: recover cleanly if a previous run left the cores wedged
os.environ.setdefault("NEURON_RT_RESET_CORES", "1")

B, T, N = 4, 64000, 588
NCORES = 8
TS = T // 2          # 32000 time steps per core shard
P = 128              # partitions
NI = TS // P         # 250 samples per partition
VW = NI + N - 1      # 837 window columns per partition
F = int(os.environ.get("K_F", "5"))       # samples per A-chunk DMA tile
BUFS = int(os.environ.get("K_BUFS", "15"))  # A-tile pool depth
NCHUNK = NI // F

_cached = {}


def _build_program():
    import concourse.bass as bass
    import concourse.tile as tile
    from concourse import bacc, mybir

    f32 = mybir.dt.float32
    nc = bacc.Bacc("TRN2", target_bir_lowering=False, debug=False,
                   num_devices=NCORES)

    a_dram = nc.dram_tensor("a_s", [TS, N], f32, kind="ExternalInput").ap()
    v_dram = nc.dram_tensor("v", [P, VW], f32, kind="ExternalInput").ap()
    r_dram = nc.dram_tensor("r", [P, NI], f32, kind="ExternalOutput").ap()

    # [TS, N] -> [P, NI, N]: partition p holds rows 250p .. 250p+249
    a_r = a_dram.rearrange("(p n) k -> p n k", p=P)

    with tile.TileContext(nc) as tc:
        with (
            tc.tile_pool(name="a", bufs=BUFS) as a_pool,
            tc.tile_pool(name="prod", bufs=2) as p_pool,
            tc.tile_pool(name="const", bufs=1) as c_pool,
        ):
            v_tile = c_pool.tile([P, VW], f32)
            v_eng = nc.scalar if os.environ.get("K_VACT", "0") == "1" else nc.sync
            v_eng.dma_start(v_tile[:], v_dram[:])
            out_tile = c_pool.tile([P, NI], f32)

            # ramp-up chunk sizes: tiny first chunks let the DVE start early
            ramp = [int(c) for c in os.environ.get("K_RAMP", "1,2,2").split(",")
                    if c.strip()]
            chunks = ramp + [F] * ((NI - sum(ramp)) // F)
            rem = NI - sum(chunks)
            if rem:
                chunks.append(rem)
            assert sum(chunks) == NI

            I = 0
            for csz in chunks:
                a_tile = a_pool.tile([P, csz * N], f32)
                nc.sync.dma_start(a_tile[:], a_r[:, I:I + csz, :])
                for i in range(csz):
                    prod = p_pool.tile([P, N], f32)
                    nc.vector.scalar_tensor_tensor(
                        out=prod[:],
                        in0=a_tile[:, i * N:(i + 1) * N],
                        scalar=1.0,
                        in1=v_tile[:, (NI - 1 - I):(NI - 1 - I) + N],
                        op0=mybir.AluOpType.mult,
                        op1=mybir.AluOpType.mult,
                        accum_out=out_tile[:, I:I + 1],
                    )
                    I += 1
            assert I == NI
            nc.sync.dma_start(r_dram[:], out_tile[:])
    nc.compile()
    return nc


def _build_raw_program():
    """Hand-pipelined raw-bass version (no TileContext): avoids the Tile
    end-of-kernel drain + all-engine barrier butterfly (~9us) and keeps the
    head minimal.  Sync engine streams A chunks (HWDGE, in-order per ring);
    chunk completions rotate over 8 semaphores (a single counter would race:
    the 16 SDMA engines drain their rings independently, so a later chunk's
    increments can land before an earlier chunk's last engine finishes)."""
    import concourse.bass as bass
    import concourse.tile as tile  # noqa: F401
    from concourse import bacc, mybir
    from contextlib import ExitStack

    f32 = mybir.dt.float32
    nc = bacc.Bacc("TRN2", target_bir_lowering=False, debug=False,
                   num_devices=NCORES)

    a_dram = nc.dram_tensor("a_s", [TS, N], f32, kind="ExternalInput").ap()
    v_dram = nc.dram_tensor("v", [P, VW], f32, kind="ExternalInput").ap()
    r_dram = nc.dram_tensor("r", [P, NI], f32, kind="ExternalOutput").ap()
    a_r = a_dram.rearrange("(p n) k -> p n k", p=P)

    ramp = [int(c) for c in os.environ.get("K_RAMP", "1,2,2").split(",")
            if c.strip()]
    chunks = ramp + [F] * ((NI - sum(ramp)) // F)
    rem = NI - sum(chunks)
    if rem:
        chunks.append(rem)
    assert sum(chunks) == NI
    nchunks = len(chunks)
    starts = [sum(chunks[:i]) for i in range(nchunks)]
    NSEM = 8

    with ExitStack() as ctx:
        v_tile = ctx.enter_context(nc.sbuf_tensor([P, VW], f32))
        a_buf = ctx.enter_context(nc.sbuf_tensor([P, BUFS * F * N], f32))
        prod = ctx.enter_context(nc.sbuf_tensor([P, N], f32))
        out_tile = ctx.enter_context(nc.sbuf_tensor([P, NI], f32))
        vsem = ctx.enter_context(nc.semaphore("vsem"))
        osem = ctx.enter_context(nc.semaphore("osem"))
        vq = ctx.enter_context(nc.semaphore("vq"))
        dsems = [ctx.enter_context(nc.semaphore(f"dq{i}")) for i in range(NSEM)]
        block = ctx.enter_context(nc.Block())

        @block.sync
        def _(sync):
            sync.dma_start(v_tile[:], v_dram[:]).then_inc(vsem, 16)
            for ci, csz in enumerate(chunks):
                if ci >= BUFS:
                    # slot reuse: DVE must have consumed chunk ci-BUFS
                    freed = ci - BUFS
                    sync.wait_ge(vq, starts[freed] + chunks[freed])
                slot = ci % BUFS
                sync.dma_start(
                    a_buf[:, slot * F * N: slot * F * N + csz * N],
                    a_r[:, starts[ci]:starts[ci] + csz, :],
                ).then_inc(dsems[ci % NSEM], 16)
            sync.wait_ge(vq, NI)
            sync.dma_start(r_dram[:], out_tile[:]).then_inc(osem, 16)
            # quiesce: all chunk DMAs + v + out confirmed complete
            for s in range(NSEM):
                uses = len(range(s, nchunks, NSEM))
                if uses:
                    sync.wait_ge(dsems[s], 16 * uses)
            sync.wait_ge(vsem, 16)
            sync.wait_ge(osem, 16)

        @block.vector
        def _(vector):
            I = 0
            for ci, csz in enumerate(chunks):
                vector.wait_ge(dsems[ci % NSEM], 16 * (ci // NSEM + 1))
                if ci == 0:
                    vector.wait_ge(vsem, 16)
                slot = ci % BUFS
                for i in range(csz):
                    nc.vector.scalar_tensor_tensor(
                        out=prod[:],
                        in0=a_buf[:, slot * F * N + i * N:
                                  slot * F * N + (i + 1) * N],
                        scalar=1.0,
                        in1=v_tile[:, (NI - 1 - I):(NI - 1 - I) + N],
                        op0=mybir.AluOpType.mult,
                        op1=mybir.AluOpType.mult,
                        accum_out=out_tile[:, I:I + 1],
                    ).then_inc(vq, 1)
                    I += 1
            assert I == NI

    nc.compile()
    return nc


def _get_program():
    if "nc" not in _cached:
        if os.environ.get("K_RAW", "0") == "1":
            _cached["nc"] = _build_raw_program()
        else:
            _cached["nc"] = _build_program()
    return _cached["nc"]


def _make_in_maps(y, A, zi):
    from numpy.lib.stride_tricks import sliding_window_view

    y_pad = np.concatenate([zi[:, ::-1], y], axis=1)  # [B, N+T]
    in_maps = []
    for c in range(NCORES):
        b, h = divmod(c, 2)
        base = h * TS
        a_s = A[b, base:base + TS, :]
        seg = y_pad[b, base:base + (TS - NI) + VW]          # [32587]
        v = sliding_window_view(seg, VW)[::NI]               # [128, 837]
        v = np.ascontiguousarray(v[:, ::-1])                 # reversed windows
        in_maps.append({"a_s": np.ascontiguousarray(a_s), "v": v})
    return in_maps


def _run(nc, in_maps, trace=False, **kw):
    from concourse.bass_utils import run_bass_kernel_spmd

    return run_bass_kernel_spmd(nc, in_maps, list(range(NCORES)),
                                trace=trace, **kw)


def kernel(y, A, zi):
    y = np.asarray(y, dtype=np.float32)
    A = np.asarray(A, dtype=np.float32)
    zi = np.asarray(zi, dtype=np.float32)

    nc = _get_program()
    res = _run(nc, _make_in_maps(y, A, zi))

    x = np.empty((B, T), dtype=np.float32)
    for c in range(NCORES):
        b, h = divmod(c, 2)
        base = h * TS
        r = res.results[c]["r"].reshape(TS)   # [128,250] -> t = 250p + I
        x[b, base:base + TS] = y[b, base:base + TS] + r
    return x



# revision 2
# speedup vs baseline: 1.0116x; 1.0116x over previous
"""Trainium2 Bass kernel for nn_DiffKS (time-varying FIR / InvertLPC forward).

Computes x[b,t] = y[b,t] + sum_k A[b,t,k] * y_padded[b, t+N-1-k]
with B=4, T=64000, N=588, y_padded = concat(flip(zi), y).

Sharding: 8 cores = 4 batches x 2 halves of T (each core: 32000 time steps).
Per-core layout: SBUF partition p owns 250 contiguous time steps
(t = 250*p + I, I in [0,250)).  The shifted-signal operand for output sample
I is a contiguous 588-column slice of a tiny per-partition reversed window
tile v[128, 837] (v[p] = reversed y_padded[250p : 250p+837] of the shard),
so no shift/Hankel materialization is needed on-device.  One fused DVE
multiply+reduce per output sample, while A streams from HBM in large
contiguous DMAs (memory-bound roofline, ~358 GB/s/core HBM limit).

bf16 mode (default): A is cast fp32->bf16 inline by the SWDGE DMA path
(HBM reads stay fp32 - unavoidable - but the DVE operands become 16-bit,
unlocking the 2x_1p DVE perf mode and halving vector-engine busy time,
which was the 211us/245us bottleneck of the fp32 variant).  The window
tile is shipped twice (shifted by one element) so every 588-wide slice
the DVE reads starts 4-byte aligned, a 2x_1p requirement.  Accumulation
stays fp32 (DVE internal precision + fp32 accum_out).
"""

import os
import sys

import numpy as np

for _p in ("/opt/trn_rl_repo", "/opt/pypackages"):
    if _p not in sys.path:
        sys.path.append(_p)

# defensive: recover cleanly if a previous run left the cores wedged
os.environ.setdefault("NEURON_RT_RESET_CORES", "1")

B, T, N = 4, 64000, 588
NCORES = 8
TS = T // 2          # 32000 time steps per core shard
P = 128              # partitions
NI = TS // P         # 250 samples per partition
VW = NI + N - 1      # 837 window columns per partition
F = int(os.environ.get("K_F", "5"))       # samples per A-chunk DMA tile
BUFS = int(os.environ.get("K_BUFS", "15"))  # A-tile pool depth
DTYPE = os.environ.get("K_DTYPE", "bf16")   # bf16 | f32
OP = os.environ.get("K_OP", "stt")          # stt | ttr

_cached = {}


def _chunks():
    ramp = [int(c) for c in os.environ.get("K_RAMP", "1,2,2").split(",")
            if c.strip()]
    chunks = ramp + [F] * ((NI - sum(ramp)) // F)
    rem = NI - sum(chunks)
    if rem:
        chunks.append(rem)
    assert sum(chunks) == NI
    return chunks


def _build_program():
    import concourse.bass as bass
    import concourse.tile as tile
    from concourse import bacc, mybir

    f32 = mybir.dt.float32
    bf16 = mybir.dt.bfloat16
    cdt = bf16 if DTYPE == "bf16" else f32
    nc = bacc.Bacc("TRN2", target_bir_lowering=False, debug=False,
                   num_devices=NCORES)

    a_dram = nc.dram_tensor("a_s", [TS, N], f32, kind="ExternalInput").ap()
    v_dram = nc.dram_tensor("v", [P, VW], cdt, kind="ExternalInput").ap()
    if DTYPE == "bf16":
        v1_dram = nc.dram_tensor("v1", [P, VW], cdt,
                                 kind="ExternalInput").ap()
    r_dram = nc.dram_tensor("r", [P, NI], f32, kind="ExternalOutput").ap()

    # [TS, N] -> [P, NI, N]: partition p holds rows 250p .. 250p+249
    a_r = a_dram.rearrange("(p n) k -> p n k", p=P)

    # bf16: A-chunk DMA casts fp32->bf16 inline (SWDGE/gpsimd only)
    a_dma_eng = nc.gpsimd if DTYPE == "bf16" else nc.sync

    with tile.TileContext(nc) as tc:
        with (
            tc.tile_pool(name="a", bufs=BUFS) as a_pool,
            tc.tile_pool(name="prod", bufs=2) as p_pool,
            tc.tile_pool(name="const", bufs=1) as c_pool,
        ):
            v_tile = c_pool.tile([P, VW], cdt)
            nc.sync.dma_start(v_tile[:], v_dram[:])
            if DTYPE == "bf16":
                v1_tile = c_pool.tile([P, VW], cdt)
                nc.sync.dma_start(v1_tile[:], v1_dram[:])
            out_tile = c_pool.tile([P, NI], f32)

            chunks = _chunks()

            I = 0
            for csz in chunks:
                a_tile = a_pool.tile([P, csz * N], cdt)
                a_dma_eng.dma_start(a_tile[:], a_r[:, I:I + csz, :])
                for i in range(csz):
                    off = NI - 1 - I
                    if DTYPE == "bf16" and off % 2 == 1:
                        vsl = v1_tile[:, off - 1:off - 1 + N]
                    else:
                        vsl = v_tile[:, off:off + N]
                    prod = p_pool.tile([P, N], cdt)
                    if OP == "ttr":
                        nc.vector.tensor_tensor_reduce(
                            out=prod[:],
                            in0=a_tile[:, i * N:(i + 1) * N],
                            in1=vsl,
                            scale=1.0,
                            scalar=0.0,
                            op0=mybir.AluOpType.mult,
                            op1=mybir.AluOpType.add,
                            accum_out=out_tile[:, I:I + 1],
                        )
                    else:
                        nc.vector.scalar_tensor_tensor(
                            out=prod[:],
                            in0=a_tile[:, i * N:(i + 1) * N],
                            scalar=1.0,
                            in1=vsl,
                            op0=mybir.AluOpType.mult,
                            op1=mybir.AluOpType.mult,
                            accum_out=out_tile[:, I:I + 1],
                        )
                    I += 1
            assert I == NI
            nc.sync.dma_start(r_dram[:], out_tile[:])
    nc.compile()
    return nc


def _get_program():
    if "nc" not in _cached:
        _cached["nc"] = _build_program()
    return _cached["nc"]


def _make_in_maps(y, A, zi):
    import ml_dtypes
    from numpy.lib.stride_tricks import sliding_window_view

    bf16 = ml_dtypes.bfloat16
    y_pad = np.concatenate([zi[:, ::-1], y], axis=1)  # [B, N+T]
    in_maps = []
    for c in range(NCORES):
        b, h = divmod(c, 2)
        base = h * TS
        a_s = A[b, base:base + TS, :]
        seg = y_pad[b, base:base + (TS - NI) + VW]          # [32587]
        v = sliding_window_view(seg, VW)[::NI]               # [128, 837]
        v = np.ascontiguousarray(v[:, ::-1])                 # reversed windows
        m = {"a_s": np.ascontiguousarray(a_s)}
        if DTYPE == "bf16":
            m["v"] = v.astype(bf16)
            v1 = np.empty_like(v)
            v1[:, :-1] = v[:, 1:]
            v1[:, -1] = 0.0
            m["v1"] = v1.astype(bf16)
        else:
            m["v"] = v
        in_maps.append(m)
    return in_maps


def _run(nc, in_maps, trace=False, **kw):
    from concourse.bass_utils import run_bass_kernel_spmd

    return run_bass_kernel_spmd(nc, in_maps, list(range(NCORES)),
                                trace=trace, **kw)


def kernel(y, A, zi):
    y = np.asarray(y, dtype=np.float32)
    A = np.asarray(A, dtype=np.float32)
    zi = np.asarray(zi, dtype=np.float32)

    nc = _get_program()
    res = _run(nc, _make_in_maps(y, A, zi))

    x = np.empty((B, T), dtype=np.float32)
    for c in range(NCORES):
        b, h = divmod(c, 2)
        base = h * TS
        r = res.results[c]["r"].reshape(TS)   # [128,250] -> t = 250p + I
        x[b, base:base + TS] = y[b, base:base + TS] + r
    return x


# revision 3
# speedup vs baseline: 1.0331x; 1.0213x over previous
"""Trainium2 Bass kernel for nn_DiffKS (time-varying FIR / InvertLPC forward).

Computes x[b,t] = y[b,t] + sum_k A[b,t,k] * y_padded[b, t+N-1-k]
with B=4, T=64000, N=588, y_padded = concat(flip(zi), y).

Sharding: 8 cores = 4 batches x 2 halves of T (each core: 32000 time steps).
Per-core layout: SBUF partition p owns 250 contiguous time steps
(t = 250*p + I, I in [0,250)).  The shifted-signal operand for output sample
I is a contiguous 588-column slice of a tiny per-partition reversed window
tile v[128, 837] (v[p] = reversed y_padded[250p : 250p+837] of the shard),
so no shift/Hankel materialization is needed on-device.  One fused DVE
multiply+reduce per output sample, while A streams from HBM in large
contiguous DMAs (memory-bound roofline, ~358 GB/s/core HBM limit).

Dtype modes (K_DTYPE):
  f32  - A stays fp32 end to end, HWDGE stream (fastest DMA path).
         K_PROD=bf16 makes the throwaway multiply output 16-bit and
         K_VBF=1 makes the window operand 16-bit - both shave DVE
         per-op time without touching the A stream.
  bf16 - A cast fp32->bf16 inline by the SWDGE DMA path; all-16-bit DVE
         ops (measured 714ns vs 845ns per op) but the SWDGE stream runs
         ~3% slower than HWDGE and pays ~1.4us Q7 emission per chunk.
Accumulation is always fp32 (DVE internal precision + fp32 accum_out).
"""

import os
import sys

import numpy as np

for _p in ("/opt/trn_rl_repo", "/opt/pypackages"):
    if _p not in sys.path:
        sys.path.append(_p)

# defensive: recover cleanly if a previous run left the cores wedged
os.environ.setdefault("NEURON_RT_RESET_CORES", "1")

B, T, N = 4, 64000, 588
NCORES = 8
TS = T // 2          # 32000 time steps per core shard
P = 128              # partitions
NI = TS // P         # 250 samples per partition
VW = NI + N - 1      # 837 window columns per partition
F = int(os.environ.get("K_F", "5"))       # samples per A-chunk DMA tile
BUFS = int(os.environ.get("K_BUFS", "15"))  # A-tile pool depth
DTYPE = os.environ.get("K_DTYPE", "f32")    # f32 | bf16 (A stream dtype)
PROD = os.environ.get("K_PROD", "bf16")     # bf16 | same (throwaway out)
VBF = os.environ.get("K_VBF", "1") == "1"   # 16-bit window operand
OP = os.environ.get("K_OP", "stt")          # stt | ttr
RAMP = os.environ.get("K_RAMP", "1,2,2")    # head chunk sizes
TAIL = os.environ.get("K_TAIL", "2,2,1")    # tail chunk sizes
SPLIT_OUT = os.environ.get("K_SPLIT_OUT", "1") == "1"

_cached = {}


def _chunks():
    ramp = [int(c) for c in RAMP.split(",") if c.strip()]
    tail = [int(c) for c in TAIL.split(",") if c.strip()]
    mid = NI - sum(ramp) - sum(tail)
    chunks = ramp + [F] * (mid // F)
    rem = mid - (mid // F) * F
    if rem:
        chunks.append(rem)
    chunks += tail
    assert sum(chunks) == NI
    return chunks


def _build_program():
    import concourse.bass as bass
    import concourse.tile as tile
    from concourse import bacc, mybir

    f32 = mybir.dt.float32
    bf16 = mybir.dt.bfloat16
    a_dt = bf16 if DTYPE == "bf16" else f32
    v_dt = bf16 if (DTYPE == "bf16" or VBF) else f32
    p_dt = bf16 if (PROD == "bf16" or DTYPE == "bf16") else a_dt
    # alignment trick (second, one-element-shifted window copy) is only
    # needed when a 2-byte window operand could engage packed DVE modes
    need_v1 = v_dt == bf16
    nc = bacc.Bacc("TRN2", target_bir_lowering=False, debug=False,
                   num_devices=NCORES)

    a_dram = nc.dram_tensor("a_s", [TS, N], f32, kind="ExternalInput").ap()
    v_dram = nc.dram_tensor("v", [P, VW], v_dt, kind="ExternalInput").ap()
    if need_v1:
        v1_dram = nc.dram_tensor("v1", [P, VW], v_dt,
                                 kind="ExternalInput").ap()
    r_dram = nc.dram_tensor("r", [P, NI], f32, kind="ExternalOutput").ap()

    # [TS, N] -> [P, NI, N]: partition p holds rows 250p .. 250p+249
    a_r = a_dram.rearrange("(p n) k -> p n k", p=P)

    # bf16 A: chunk DMA casts fp32->bf16 inline (SWDGE/gpsimd only)
    a_dma_eng = nc.gpsimd if DTYPE == "bf16" else nc.sync

    with tile.TileContext(nc) as tc:
        with (
            tc.tile_pool(name="a", bufs=BUFS) as a_pool,
            tc.tile_pool(name="prod", bufs=2) as p_pool,
            tc.tile_pool(name="const", bufs=1) as c_pool,
        ):
            v_tile = c_pool.tile([P, VW], v_dt)
            nc.sync.dma_start(v_tile[:], v_dram[:])
            if need_v1:
                v1_tile = c_pool.tile([P, VW], v_dt)
                nc.sync.dma_start(v1_tile[:], v1_dram[:])
            out_tile = c_pool.tile([P, NI], f32)

            chunks = _chunks()
            half = NI // 2

            I = 0
            stored = 0
            for csz in chunks:
                a_tile = a_pool.tile([P, csz * N], a_dt)
                a_dma_eng.dma_start(a_tile[:], a_r[:, I:I + csz, :])
                for i in range(csz):
                    off = NI - 1 - I
                    if need_v1 and off % 2 == 1:
                        vsl = v1_tile[:, off - 1:off - 1 + N]
                    else:
                        vsl = v_tile[:, off:off + N]
                    prod = p_pool.tile([P, N], p_dt)
                    if OP == "ttr":
                        nc.vector.tensor_tensor_reduce(
                            out=prod[:],
                            in0=a_tile[:, i * N:(i + 1) * N],
                            in1=vsl,
                            scale=1.0,
                            scalar=0.0,
                            op0=mybir.AluOpType.mult,
                            op1=mybir.AluOpType.add,
                            accum_out=out_tile[:, I:I + 1],
                        )
                    else:
                        nc.vector.scalar_tensor_tensor(
                            out=prod[:],
                            in0=a_tile[:, i * N:(i + 1) * N],
                            scalar=1.0,
                            in1=vsl,
                            op0=mybir.AluOpType.mult,
                            op1=mybir.AluOpType.mult,
                            accum_out=out_tile[:, I:I + 1],
                        )
                    I += 1
                if SPLIT_OUT and stored == 0 and I >= half:
                    # first half of the result leaves early so the final
                    # store is tiny and the tail doesn't trail the stream
                    nc.sync.dma_start(r_dram[:, :I], out_tile[:, :I])
                    stored = I
            assert I == NI
            nc.sync.dma_start(r_dram[:, stored:], out_tile[:, stored:])
    nc.compile()
    return nc


def _get_program():
    if "nc" not in _cached:
        _cached["nc"] = _build_program()
    return _cached["nc"]


def _make_in_maps(y, A, zi):
    import ml_dtypes
    from numpy.lib.stride_tricks import sliding_window_view

    bf16 = ml_dtypes.bfloat16
    v_bf = DTYPE == "bf16" or VBF
    y_pad = np.concatenate([zi[:, ::-1], y], axis=1)  # [B, N+T]
    in_maps = []
    for c in range(NCORES):
        b, h = divmod(c, 2)
        base = h * TS
        a_s = A[b, base:base + TS, :]
        seg = y_pad[b, base:base + (TS - NI) + VW]          # [32587]
        v = sliding_window_view(seg, VW)[::NI]               # [128, 837]
        v = np.ascontiguousarray(v[:, ::-1])                 # reversed windows
        m = {"a_s": np.ascontiguousarray(a_s)}
        if v_bf:
            m["v"] = v.astype(bf16)
            v1 = np.empty_like(v)
            v1[:, :-1] = v[:, 1:]
            v1[:, -1] = 0.0
            m["v1"] = v1.astype(bf16)
        else:
            m["v"] = v
        in_maps.append(m)
    return in_maps


def _run(nc, in_maps, trace=False, **kw):
    from concourse.bass_utils import run_bass_kernel_spmd

    return run_bass_kernel_spmd(nc, in_maps, list(range(NCORES)),
                                trace=trace, **kw)


def kernel(y, A, zi):
    y = np.asarray(y, dtype=np.float32)
    A = np.asarray(A, dtype=np.float32)
    zi = np.asarray(zi, dtype=np.float32)

    nc = _get_program()
    res = _run(nc, _make_in_maps(y, A, zi))

    x = np.empty((B, T), dtype=np.float32)
    for c in range(NCORES):
        b, h = divmod(c, 2)
        base = h * TS
        r = res.results[c]["r"].reshape(TS)   # [128,250] -> t = 250p + I
        x[b, base:base + TS] = y[b, base:base + TS] + r
    return x


# revision 10
# speedup vs baseline: 1.1819x; 1.1441x over previous
"""Trainium2 Bass kernel for nn_DiffKS (time-varying FIR / InvertLPC forward).

Computes x[b,t] = y[b,t] + sum_k A[b,t,k] * y_padded[b, t+N-1-k]
with B=4, T=64000, N=588, y_padded = concat(flip(zi), y).

Sharding: 8 cores = 4 batches x 2 halves of T (each core: 32000 time steps).
Per-core layout: SBUF partition p owns 250 contiguous time steps
(t = 250*p + I, I in [0,250)).  The shifted-signal operand for output sample
I is a contiguous 588-column slice of a tiny per-partition reversed window
tile v[128, 837] (v[p] = reversed y_padded[250p : 250p+837] of the shard),
so no shift/Hankel materialization is needed on-device.  One fused DVE
multiply+reduce per output sample, while A streams from HBM in large
contiguous DMAs (memory-bound roofline, ~358 GB/s/core HBM limit).

Dtype modes (K_DTYPE):
  f32  - A stays fp32 end to end, HWDGE stream (fastest DMA path).
         K_PROD=bf16 makes the throwaway multiply output 16-bit and
         K_VBF=1 makes the window operand 16-bit - both shave DVE
         per-op time without touching the A stream.
  bf16 - A cast fp32->bf16 inline by the SWDGE DMA path; all-16-bit DVE
         ops (measured 714ns vs 845ns per op) but the SWDGE stream runs
         ~3% slower than HWDGE and pays ~1.4us Q7 emission per chunk.
Accumulation is always fp32 (DVE internal precision + fp32 accum_out).
"""

import os
import sys

import numpy as np

for _p in ("/opt/trn_rl_repo", "/opt/pypackages"):
    if _p not in sys.path:
        sys.path.append(_p)

# defensive: recover cleanly if a previous run left the cores wedged
os.environ.setdefault("NEURON_RT_RESET_CORES", "1")

B, T, N = 4, 64000, 588
NCORES = 8
TS = T // 2          # 32000 time steps per core shard
P = 128              # partitions
NI = TS // P         # 250 samples per partition
VW = NI + N - 1      # 837 window columns per partition
F = int(os.environ.get("K_F", "5"))       # samples per A-chunk DMA tile
BUFS = int(os.environ.get("K_BUFS", "15"))  # A-tile pool depth
DTYPE = os.environ.get("K_DTYPE", "f32")    # f32 | bf16 (A stream dtype)
PROD = os.environ.get("K_PROD", "bf16")     # bf16 | same (throwaway out)
VBF = os.environ.get("K_VBF", "1") == "1"   # 16-bit window operand
OP = os.environ.get("K_OP", "stt")          # stt | ttr
RAMP = os.environ.get("K_RAMP", "")         # head chunk sizes ("" = none)
TAIL = os.environ.get("K_TAIL", "2,1")      # tail chunk sizes
SPLIT_OUT = os.environ.get("K_SPLIT_OUT", "1") == "1"
ALT = os.environ.get("K_ALT", "1") == "1"   # alternate both HWDGE rings

_cached = {}


def _chunks():
    ramp = [int(c) for c in RAMP.split(",") if c.strip()]
    tail = [int(c) for c in TAIL.split(",") if c.strip()]
    mid = NI - sum(ramp) - sum(tail)
    chunks = ramp + [F] * (mid // F)
    rem = mid - (mid // F) * F
    if rem:
        chunks.append(rem)
    chunks += tail
    assert sum(chunks) == NI
    return chunks


def _build_program():
    import concourse.bass as bass
    import concourse.tile as tile
    from concourse import bacc, mybir

    f32 = mybir.dt.float32
    bf16 = mybir.dt.bfloat16
    a_dt = bf16 if DTYPE == "bf16" else f32
    v_dt = bf16 if (DTYPE == "bf16" or VBF) else f32
    p_dt = bf16 if (PROD == "bf16" or DTYPE == "bf16") else a_dt
    # alignment trick (second, one-element-shifted window copy) is only
    # needed when a 2-byte window operand could engage packed DVE modes
    need_v1 = v_dt == bf16
    nc = bacc.Bacc("TRN2", target_bir_lowering=False, debug=False,
                   num_devices=NCORES)

    a_dram = nc.dram_tensor("a_s", [TS, N], f32, kind="ExternalInput").ap()
    v_dram = nc.dram_tensor("v", [P, VW], v_dt, kind="ExternalInput").ap()
    if need_v1:
        v1_dram = nc.dram_tensor("v1", [P, VW], v_dt,
                                 kind="ExternalInput").ap()
    r_dram = nc.dram_tensor("r", [P, NI], f32, kind="ExternalOutput").ap()

    # [TS, N] -> [P, NI, N]: partition p holds rows 250p .. 250p+249
    a_r = a_dram.rearrange("(p n) k -> p n k", p=P)

    # bf16 A: chunk DMA casts fp32->bf16 inline (SWDGE/gpsimd only).
    # f32: alternate the two HWDGE rings (qSPDynamicHW / qActDynamicHW)
    # so descriptor-completion round trips on one ring don't stall the
    # stream.
    if DTYPE == "bf16":
        a_engs = [nc.gpsimd]
    elif ALT:
        a_engs = [nc.sync, nc.scalar]
    else:
        a_engs = [nc.sync]

    with tile.TileContext(nc) as tc:
        with (
            tc.tile_pool(name="a", bufs=BUFS) as a_pool,
            tc.tile_pool(name="prod", bufs=2) as p_pool,
            tc.tile_pool(name="const", bufs=1) as c_pool,
        ):
            # window tiles ride the second HWDGE ring so the A stream's
            # first chunk issues immediately on the first ring
            v_eng = nc.scalar if ALT and DTYPE != "bf16" else nc.sync
            v_tile = c_pool.tile([P, VW], v_dt)
            v_eng.dma_start(v_tile[:], v_dram[:])
            if need_v1:
                v1_tile = c_pool.tile([P, VW], v_dt)
                v_eng.dma_start(v1_tile[:], v1_dram[:])
            out_tile = c_pool.tile([P, NI], f32)

            chunks = _chunks()
            half = NI // 2

            I = 0
            stored = 0
            for ci, csz in enumerate(chunks):
                a_tile = a_pool.tile([P, csz * N], a_dt)
                a_engs[ci % len(a_engs)].dma_start(
                    a_tile[:], a_r[:, I:I + csz, :])
                for i in range(csz):
                    off = NI - 1 - I
                    if need_v1 and off % 2 == 1:
                        vsl = v1_tile[:, off - 1:off - 1 + N]
                    else:
                        vsl = v_tile[:, off:off + N]
                    prod = p_pool.tile([P, N], p_dt)
                    if OP == "ttr":
                        nc.vector.tensor_tensor_reduce(
                            out=prod[:],
                            in0=a_tile[:, i * N:(i + 1) * N],
                            in1=vsl,
                            scale=1.0,
                            scalar=0.0,
                            op0=mybir.AluOpType.mult,
                            op1=mybir.AluOpType.add,
                            accum_out=out_tile[:, I:I + 1],
                        )
                    else:
                        nc.vector.scalar_tensor_tensor(
                            out=prod[:],
                            in0=a_tile[:, i * N:(i + 1) * N],
                            scalar=1.0,
                            in1=vsl,
                            op0=mybir.AluOpType.mult,
                            op1=mybir.AluOpType.mult,
                            accum_out=out_tile[:, I:I + 1],
                        )
                    I += 1
                if SPLIT_OUT and stored == 0 and I >= half:
                    # first half of the result leaves early so the final
                    # store is tiny and the tail doesn't trail the stream
                    nc.sync.dma_start(r_dram[:, :I], out_tile[:, :I])
                    stored = I
            assert I == NI
            nc.sync.dma_start(r_dram[:, stored:], out_tile[:, stored:])
    nc.compile()
    return nc


def _build_raw_program():
    """Hand-pipelined raw-bass version (no TileContext).

    Two HWDGE rings stream A concurrently: sync (qSPDynamicHW) issues
    even chunks, scalar (qActDynamicHW) odd chunks + the window tiles.
    With one ring, the issuing engine's slot-reuse waits serialize with
    its dma_starts and the ring drains dry whenever the DVE lags; with
    two rings the other ring keeps the SDMA engines fed (measured 413
    GB/s vs 340 single-ring).  Chunk completions rotate over 8
    semaphores per ring (the 16 SDMA engines drain independently, so a
    later chunk's increments can land before an earlier chunk's last
    engine finishes; rotation depth 8 > max chunks in flight per ring).
    """
    import concourse.bass as bass
    from concourse import bacc, mybir
    from contextlib import ExitStack

    f32 = mybir.dt.float32
    bf16 = mybir.dt.bfloat16
    a_dt = f32
    v_dt = bf16 if VBF else f32
    p_dt = bf16 if PROD == "bf16" else f32
    need_v1 = v_dt == bf16
    nc = bacc.Bacc("TRN2", target_bir_lowering=False, debug=False,
                   num_devices=NCORES)

    a_dram = nc.dram_tensor("a_s", [TS, N], f32, kind="ExternalInput").ap()
    v_dram = nc.dram_tensor("v", [P, VW], v_dt, kind="ExternalInput").ap()
    if need_v1:
        v1_dram = nc.dram_tensor("v1", [P, VW], v_dt,
                                 kind="ExternalInput").ap()
    r_dram = nc.dram_tensor("r", [P, NI], f32, kind="ExternalOutput").ap()
    a_r = a_dram.rearrange("(p n) k -> p n k", p=P)

    chunks = _chunks()
    nchunks = len(chunks)
    starts = [sum(chunks[:i]) for i in range(nchunks)]
    NSEM = 8
    NRING = 2 if ALT else 1
    nv = 2 if need_v1 else 1
    half = NI // 2

    with ExitStack() as ctx:
        v_tile = ctx.enter_context(nc.sbuf_tensor([P, VW], v_dt))
        if need_v1:
            v1_tile = ctx.enter_context(nc.sbuf_tensor([P, VW], v_dt))
        a_buf = ctx.enter_context(nc.sbuf_tensor([P, BUFS * F * N], a_dt))
        prod = ctx.enter_context(nc.sbuf_tensor([P, N], p_dt))
        out_tile = ctx.enter_context(nc.sbuf_tensor([P, NI], f32))
        vsem = ctx.enter_context(nc.semaphore("vsem"))
        osem = ctx.enter_context(nc.semaphore("osem"))
        vq = ctx.enter_context(nc.semaphore("vq"))
        dsems = [[ctx.enter_context(nc.semaphore(f"dq{r}_{i}"))
                  for i in range(NSEM)] for r in range(NRING)]
        block = ctx.enter_context(nc.Block())

        def issue_ring(eng, ring):
            cis = list(range(ring, nchunks, NRING))
            for j, ci in enumerate(cis):
                if ci >= BUFS:
                    # slot reuse: DVE must have consumed chunk ci-BUFS
                    freed = ci - BUFS
                    eng.wait_ge(vq, starts[freed] + chunks[freed])
                slot = ci % BUFS
                csz = chunks[ci]
                eng.dma_start(
                    a_buf[:, slot * F * N: slot * F * N + csz * N],
                    a_r[:, starts[ci]:starts[ci] + csz, :],
                ).then_inc(dsems[ring][j % NSEM], 16)

        @block.sync
        def _(sync):
            if NRING == 1:
                sync.dma_start(v_tile[:], v_dram[:]).then_inc(vsem, 16)
                if need_v1:
                    sync.dma_start(v1_tile[:],
                                   v1_dram[:]).then_inc(vsem, 16)
            issue_ring(sync, 0)
            if SPLIT_OUT:
                sync.wait_ge(vq, half)
                sync.dma_start(r_dram[:, :half],
                               out_tile[:, :half]).then_inc(osem, 16)
            sync.wait_ge(vq, NI)
            sync.dma_start(r_dram[:, half if SPLIT_OUT else 0:],
                           out_tile[:, half if SPLIT_OUT else 0:],
                           ).then_inc(osem, 16)
            # quiesce: all chunk DMAs + v + out confirmed complete
            for r in range(NRING):
                n_ring = len(range(r, nchunks, NRING))
                for s in range(NSEM):
                    uses = len(range(s, n_ring, NSEM))
                    if uses:
                        sync.wait_ge(dsems[r][s], 16 * uses)
            sync.wait_ge(vsem, 16 * nv)
            sync.wait_ge(osem, 32 if SPLIT_OUT else 16)

        if NRING > 1:
            @block.scalar
            def _(scalar):
                scalar.dma_start(v_tile[:], v_dram[:]).then_inc(vsem, 16)
                if need_v1:
                    scalar.dma_start(v1_tile[:],
                                     v1_dram[:]).then_inc(vsem, 16)
                issue_ring(scalar, 1)

        @block.vector
        def _(vector):
            vector.wait_ge(vsem, 16 * nv)
            I = 0
            for ci, csz in enumerate(chunks):
                r = ci % NRING
                j = ci // NRING
                vector.wait_ge(dsems[r][j % NSEM], 16 * (j // NSEM + 1))
                slot = ci % BUFS
                for i in range(csz):
                    off = NI - 1 - I
                    if need_v1 and off % 2 == 1:
                        vsl = v1_tile[:, off - 1:off - 1 + N]
                    else:
                        vsl = v_tile[:, off:off + N]
                    nc.vector.scalar_tensor_tensor(
                        out=prod[:],
                        in0=a_buf[:, slot * F * N + i * N:
                                  slot * F * N + (i + 1) * N],
                        scalar=1.0,
                        in1=vsl,
                        op0=mybir.AluOpType.mult,
                        op1=mybir.AluOpType.mult,
                        accum_out=out_tile[:, I:I + 1],
                    ).then_inc(vq, 1)
                    I += 1
            assert I == NI

    nc.compile()
    return nc


def _get_program():
    if "nc" not in _cached:
        if os.environ.get("K_RAW", "1") == "1":
            _cached["nc"] = _build_raw_program()
        else:
            _cached["nc"] = _build_program()
    return _cached["nc"]


def _make_in_maps(y, A, zi):
    import ml_dtypes
    from numpy.lib.stride_tricks import sliding_window_view

    bf16 = ml_dtypes.bfloat16
    v_bf = DTYPE == "bf16" or VBF
    y_pad = np.concatenate([zi[:, ::-1], y], axis=1)  # [B, N+T]
    in_maps = []
    for c in range(NCORES):
        b, h = divmod(c, 2)
        base = h * TS
        a_s = A[b, base:base + TS, :]
        seg = y_pad[b, base:base + (TS - NI) + VW]          # [32587]
        v = sliding_window_view(seg, VW)[::NI]               # [128, 837]
        v = np.ascontiguousarray(v[:, ::-1])                 # reversed windows
        m = {"a_s": np.ascontiguousarray(a_s)}
        if v_bf:
            m["v"] = v.astype(bf16)
            v1 = np.empty_like(v)
            v1[:, :-1] = v[:, 1:]
            v1[:, -1] = 0.0
            m["v1"] = v1.astype(bf16)
        else:
            m["v"] = v
        in_maps.append(m)
    return in_maps


def _run(nc, in_maps, trace=False, **kw):
    from concourse.bass_utils import run_bass_kernel_spmd

    return run_bass_kernel_spmd(nc, in_maps, list(range(NCORES)),
                                trace=trace, **kw)


def kernel(y, A, zi):
    y = np.asarray(y, dtype=np.float32)
    A = np.asarray(A, dtype=np.float32)
    zi = np.asarray(zi, dtype=np.float32)

    nc = _get_program()
    res = _run(nc, _make_in_maps(y, A, zi))

    x = np.empty((B, T), dtype=np.float32)
    for c in range(NCORES):
        b, h = divmod(c, 2)
        base = h * TS
        r = res.results[c]["r"].reshape(TS)   # [128,250] -> t = 250p + I
        x[b, base:base + TS] = y[b, base:base + TS] + r
    return x
